# revision 1
# baseline (speedup 1.0000x reference)
"""Trainium2 Bass kernel for nn_MultiHeadAttention (B=2, S=2048, E=1024, H=16).

Sharding: 8 NeuronCores = data-parallel over the 2 batches x tensor-parallel
over the 16 heads in 4 groups of 4 heads (Wq/Wk/Wv split column-wise, Wo
row-wise).  Each core computes a full-[S, E] partial of its batch's output;
the host sums the 4 head-group partials per batch.

Per-core device algorithm (S.T orientation so exp(S.T) feeds P@V directly):
  Q.T/K.T[n, s] = (wT chunk).T @ xT chunk      (e-outer, chases input DMAs)
  V[s, n]       = (xvT chunk).T @ wvT chunk    stored as v_ext = [V_h | ones]
  S.T_h[k, q]   = (K_h.T chunk).T @ Q_h.T      row-packed head pairs (d=64)
  P.T           = exp(S.T / 8)                 one ACT op per (k, head pair)
  [O.T_h; sums] = (v_ext_h).T @ P.T_h          fused: PSUM rows 0-63 = O.T_h,
                                               rows 64-127 = rowsum broadcast
  O.Tn_h        = O.T_h * recip(sums)          recip shifted p64->p0 via DMA
  out[m, :]     = sum_h (oT_h chunk).T @ woT_h

dtypes: matmul inputs for the projections are fp16 (host pre-cast halves the
HBM traffic; 10-bit mantissa beats bf16 by 8x); everything SBUF-internal
(Q.T/K.T/V/P.T/O.T/Wo) is float32r (full fp32 bits, reduced-precision
multiply); accumulation is always fp32.
"""

import numpy as np
from contextlib import ExitStack

import ml_dtypes

import concourse.bass as bass
import concourse.mybir as mybir
import concourse.tile as tile
from concourse.tile import ScopedClock
from concourse.bass_utils import run_bass_kernel_spmd

# ---------------------------------------------------------------------------
# Workarounds for the walrus build on this stack, which rejects more than ONE
# semaphore wait per instruction ("Too many sync wait commands").
# ---------------------------------------------------------------------------
_orig_commit_instruction = tile.TileContext._commit_instruction


def _commit_instruction(self, inst, lazy_reg_writes=True):
    si = getattr(inst, "sync_info", None)
    if si is not None and si.on_wait and len(si.on_wait) > 1:
        waits = list(si.on_wait)
        for w in waits[:-1]:
            nop = mybir.InstNoOp(
                name=self.nc.get_next_instruction_name(),
                ins=[], outs=[], engine=inst.engine,
            )
            nop.bass_nofuse = True
            nop.sync_info = mybir.SyncInfo(on_wait=[w], on_update=[])
            _orig_commit_instruction(self, nop, lazy_reg_writes=False)
        inst.sync_info = mybir.SyncInfo(
            on_wait=[waits[-1]], on_update=list(si.on_update or [])
        )
    return _orig_commit_instruction(self, inst, lazy_reg_writes)


def _drain_and_barrier(self, tick_clock, wait_clock):
    nc = self.nc
    drain_inst = nc.sync.drain()
    wait_clock.add_sem_waits(
        drain_inst.ins, ScopedClock({None: tick_clock.global_clock})
    )
    si = drain_inst.ins.sync_info
    waits = list(si.on_wait) if si and si.on_wait else []
    if len(waits) > 1:
        drain_inst.ins.sync_info = mybir.SyncInfo(
            on_wait=waits[:1], on_update=list(si.on_update or [])
        )
        for w in waits[1:]:
            extra = nc.sync.drain()
            esi = extra.ins.sync_info
            extra.ins.sync_info = mybir.SyncInfo(
                on_wait=[w],
                on_update=list(esi.on_update or []) if esi else [],
            )
    nc.all_engine_barrier()
    assert self.sems is not None
    popped = nc._tile_sem_poison_stack.pop()
    assert popped is self._sem_poison
    nc.clear_and_free_semaphores(list(self.sems.allocated().values()))
    nc.all_engine_barrier()


def _apply_tilefix():
    tile.TileContext._commit_instruction = _commit_instruction
    tile.TileContext._drain_and_barrier = _drain_and_barrier


_apply_tilefix()

# ---------------------------------------------------------------------------
# Problem constants (hardcoded)
# ---------------------------------------------------------------------------
B, S, E, H = 2, 2048, 1024, 16
HC, D = 4, 64              # heads per core, head dim
NCORES = 8
NE = E // 128              # 8  e-chunks
NQ = S // 512              # 4  q-chunks
NK = S // 128              # 16 k-chunks
NM = S // 128              # 16 m-chunks

F32 = mybir.dt.float32
BF16 = mybir.dt.bfloat16
FP16 = mybir.dt.float16


def build(mmdt=mybir.dt.float32r, pdt=mybir.dt.float32r, xdt=FP16,
          ovbufs=3, xbufs=1, shift_eng="scalar", ptbufs=6):
    nc = bass.Bass()
    xqT = nc.dram_tensor("xqT", [E, S], xdt, kind="ExternalInput")
    xkT = nc.dram_tensor("xkT", [E, S], xdt, kind="ExternalInput")
    xvT = nc.dram_tensor("xvT", [E, S], xdt, kind="ExternalInput")
    wqT = nc.dram_tensor("wqT", [E, 256], xdt, kind="ExternalInput")
    wkT = nc.dram_tensor("wkT", [E, 256], xdt, kind="ExternalInput")
    wvT = nc.dram_tensor("wvT", [E, 256], xdt, kind="ExternalInput")
    woT = nc.dram_tensor("woT", [256, E], mmdt, kind="ExternalInput")
    vones = nc.dram_tensor("vones", [128, 256], mmdt, kind="ExternalInput")
    out = nc.dram_tensor("out", [S, E], F32, kind="ExternalOutput")

    with tile.TileContext(nc) as tc, ExitStack() as ctx:
        consts = ctx.enter_context(tc.tile_pool(name="consts", bufs=1))
        wpool = ctx.enter_context(tc.tile_pool(name="w", bufs=1))
        actpool = ctx.enter_context(tc.tile_pool(name="acts", bufs=1))
        xpool = ctx.enter_context(tc.tile_pool(name="x", bufs=10))

        # preload the exp table before the hot loop
        dummy = consts.tile([1, 8], F32)
        nc.vector.memset(dummy[:], 0.0)
        nc.scalar.activation(dummy[:], dummy[:], mybir.ActivationFunctionType.Exp)

        wv_sb = wpool.tile([128, NE, 256], xdt)
        wo_sb = wpool.tile([64, HC, E], mmdt)

        qT_sb = actpool.tile([128, 2, S], mmdt)        # [(2 heads x d), pair, s]
        kT_sb = actpool.tile([128, 2, S], mmdt)
        v_sb = actpool.tile([128, NK, HC, 128], mmdt)  # [s%128, k, h, V_h|ones]

        def proj_eouter(w_sb, xchunks, dst, psA):
            tiles = [psA.tile([128, 512], F32, tag="mm", name=f"pj{i}")
                     for i in range(8)]
            for e in range(NE):
                for nch in range(2):
                    for m in range(NQ):
                        nc.tensor.matmul(
                            tiles[nch * NQ + m][:],
                            w_sb[:, e, nch * 128:(nch + 1) * 128],
                            xchunks[e][:, m * 512:(m + 1) * 512],
                            start=(e == 0), stop=(e == NE - 1),
                        )
            for nch in range(2):
                for m in range(NQ):
                    nc.vector.tensor_copy(
                        dst[:, nch, m * 512:(m + 1) * 512],
                        tiles[nch * NQ + m][:])

        # ---- prefix: K then Q projections (e-outer, DMA-chasing) ----
        with tc.tile_pool(name="wprefix", bufs=1) as wprefix, \
             tc.tile_pool(name="psA", bufs=8, space="PSUM") as psA:
            wk_sb = wprefix.tile([128, NE, 256], xdt)
            wq_sb = wprefix.tile([128, NE, 256], xdt)
            nc.sync.dma_start(wk_sb[:], wkT.rearrange("(ec p) n -> p ec n", p=128))
            nc.sync.dma_start(wq_sb[:], wqT.rearrange("(ec p) n -> p ec n", p=128))

            xk = []
            for e in range(NE):
                t = xpool.tile([128, S], xdt, tag="xchunk", name=f"xk{e}")
                nc.sync.dma_start(t[:], xkT[e * 128:(e + 1) * 128, :])
                xk.append(t)
            xq = []
            for e in range(NE):
                t = xpool.tile([128, S], xdt, tag="xchunk", name=f"xq{e}")
                nc.sync.dma_start(t[:], xqT[e * 128:(e + 1) * 128, :])
                xq.append(t)

            proj_eouter(wk_sb, xk, kT_sb, psA)
            proj_eouter(wq_sb, xq, qT_sb, psA)

        # V-side loads stream in behind the prefix on the SP queue
        nc.sync.dma_start(wv_sb[:], wvT.rearrange("(ec p) n -> p ec n", p=128))
        nc.sync.dma_start(wo_sb[:], woT.rearrange("(h p) j -> p h j", p=64))
        for k in range(NK):
            nc.gpsimd.dma_start(
                v_sb[:, k, :, 64:128],
                vones.rearrange("p (h c) -> p h c", h=HC))
        xv = []
        for e in range(NE):
            t = xpool.tile([128, S], xdt, tag="xchunk", name=f"xv{e}")
            nc.sync.dma_start(t[:], xvT[e * 128:(e + 1) * 128, :])
            xv.append(t)

        # ---- steady state pools ----
        oTpool = ctx.enter_context(tc.tile_pool(name="oT", bufs=1))
        ppool = ctx.enter_context(tc.tile_pool(name="pT", bufs=ptbufs))
        rpool = ctx.enter_context(tc.tile_pool(name="recip", bufs=2))
        opool = ctx.enter_context(tc.tile_pool(name="outstage", bufs=2))
        psS = ctx.enter_context(tc.tile_pool(name="psS", bufs=2, space="PSUM"))
        psOV = ctx.enter_context(tc.tile_pool(name="psOV", bufs=ovbufs, space="PSUM"))
        psX = ctx.enter_context(tc.tile_pool(name="psX", bufs=xbufs, space="PSUM"))

        oT_sb = oTpool.tile([64, HC, S], mmdt)         # [d, h, s]

        def v_proj_tile(m):
            ps = psX.tile([128, 512], F32, tag="px", name=f"vp{m}")
            for e in range(NE):
                nc.tensor.matmul(
                    ps[:, 0:256],
                    xv[e][:, m * 128:(m + 1) * 128],
                    wv_sb[:, e, :],
                    start=(e == 0), stop=(e == NE - 1),
                )
            nc.vector.tensor_copy(
                v_sb[:, m, :, 0:64],
                ps[:, 0:256].rearrange("p (h c) -> p h c", h=HC))

        def out_proj_tile(m):
            stage = opool.tile([128, E], F32)
            for j in range(2):
                ps = psX.tile([128, 512], F32, tag="px", name=f"op{m}_{j}")
                for h in range(HC):
                    nc.tensor.matmul(
                        ps[:],
                        oT_sb[:, h, m * 128:(m + 1) * 128],
                        wo_sb[:, h, j * 512:(j + 1) * 512],
                        start=(h == 0), stop=(h == HC - 1),
                    )
                nc.vector.tensor_copy(stage[:, j * 512:(j + 1) * 512], ps[:])
            nc.gpsimd.dma_start(out[m * 128:(m + 1) * 128, :], stage[:])

        # V tiles are needed from the very first pass: emit them first
        for m in range(NM):
            v_proj_tile(m)

        for qc in range(NQ):
            qs = slice(qc * 512, (qc + 1) * 512)
            for pair in range(2):
                ps_ov = [psOV.tile([128, 512], F32, name=f"ov{i}", tag="ov")
                         for i in range(2)]
                for k in range(NK):
                    ks = slice(k * 128, (k + 1) * 128)
                    first, last = (k == 0), (k == NK - 1)
                    ps_s = psS.tile([128, 1024], F32)
                    # scores, row-packed: head A rows 0-63, head B rows 64-127
                    nc.tensor.matmul(ps_s[:, 0:512],
                                     kT_sb[0:64, pair, ks],
                                     qT_sb[0:64, pair, qs],
                                     start=True, stop=True)
                    nc.tensor.matmul(ps_s[:, 512:1024],
                                     kT_sb[64:128, pair, ks],
                                     qT_sb[64:128, pair, qs],
                                     start=True, stop=True)
                    # exp of both heads in one op; 1/sqrt(D) folded into scale
                    pT = ppool.tile([128, 1024], pdt)
                    nc.scalar.activation(pT[:], ps_s[:],
                                         mybir.ActivationFunctionType.Exp,
                                         scale=0.125)
                    # fused O.T + rowsum accumulation per head
                    for h2 in range(2):
                        h = pair * 2 + h2
                        nc.tensor.matmul(
                            ps_ov[h2][:],
                            v_sb[:, k, h, :],
                            pT[:, h2 * 512:(h2 + 1) * 512],
                            start=first, stop=last)
                # normalize: recip of sums (rows 64-127), shift to rows 0-63
                for h2 in range(2):
                    h = pair * 2 + h2
                    rt = rpool.tile([128, 512], F32, tag="rt")
                    nc.vector.reciprocal(rt[64:128, :], ps_ov[h2][64:128, :])
                    rb = rpool.tile([64, 512], F32, tag="rb")
                    getattr(nc, shift_eng).dma_start(rb[:], rt[64:128, :])
                    nc.vector.tensor_tensor(
                        oT_sb[:, h, qs], ps_ov[h2][0:64, :], rb[:],
                        mybir.AluOpType.mult)
            # out-proj for this q window (needs both pairs of this qc)
            for m in range(qc * 4, qc * 4 + 4):
                out_proj_tile(m)

    return nc


_NC_CACHE = {}


def _get_nc():
    if "nc" not in _NC_CACHE:
        _NC_CACHE["nc"] = build()
    return _NC_CACHE["nc"]


def _shard_inputs(query, key, value, Wq, Wk, Wv, Wo):
    """Host-side sharding + layout prep: core c = (batch c//4, head-group c%4)."""
    f16 = np.float16
    xT = []
    for b in range(B):
        xT.append((
            np.ascontiguousarray(query[b].T).astype(f16),
            np.ascontiguousarray(key[b].T).astype(f16),
            np.ascontiguousarray(value[b].T).astype(f16),
        ))
    wT = []
    for g in range(4):
        gc = slice(g * 256, (g + 1) * 256)
        wT.append((
            np.ascontiguousarray(Wq[gc].T).astype(f16),
            np.ascontiguousarray(Wk[gc].T).astype(f16),
            np.ascontiguousarray(Wv[gc].T).astype(f16),
            np.ascontiguousarray(Wo[:, gc].T),
        ))
    vones = np.ones((128, 256), dtype=np.float32)
    in_maps = []
    for c in range(NCORES):
        b, g = c // 4, c % 4
        qT, kT, vT = xT[b]
        wq, wk, wv, wo = wT[g]
        in_maps.append({
            "xqT": qT, "xkT": kT, "xvT": vT,
            "wqT": wq, "wkT": wk, "wvT": wv, "woT": wo,
            "vones": vones,
        })
    return in_maps


def kernel(query, key, value, Wq, Wk, Wv, Wo):
    query = np.asarray(query, dtype=np.float32)
    key = np.asarray(key, dtype=np.float32)
    value = np.asarray(value, dtype=np.float32)
    Wq = np.asarray(Wq, dtype=np.float32)
    Wk = np.asarray(Wk, dtype=np.float32)
    Wv = np.asarray(Wv, dtype=np.float32)
    Wo = np.asarray(Wo, dtype=np.float32)

    nc = _get_nc()
    in_maps = _shard_inputs(query, key, value, Wq, Wk, Wv, Wo)
    res = run_bass_kernel_spmd(nc, in_maps, core_ids=list(range(NCORES)))

    out = np.zeros((B, S, E), dtype=np.float32)
    for c in range(NCORES):
        out[c // 4] += res.results[c]["out"]
    return out



# revision 4
# speedup vs baseline: 1.3583x; 1.3583x over previous
"""Trainium2 Bass kernel for nn_MultiHeadAttention (B=2, S=2048, E=1024, H=16).

Sharding: 8 NeuronCores = data-parallel over the 2 batches x tensor-parallel
over the 16 heads in 4 groups of 4 heads (Wq/Wk/Wv split column-wise, Wo
row-wise).  Each core computes a full-[S, E] partial of its batch's output;
the host sums the 4 head-group partials per batch.

Per-core pipeline (ACT exp stream is the pace-setter at ~133us):
  Q.T/K.T[n, s]  per 512-col window: psX-accumulated e-outer matmuls
                 chasing column-block DMAs (first exp at ~11us).
  S.T_h[k, q]    [64,128]x[64,512] per (window, head-pair, k-chunk),
                 head pair packed in one [128,1024] PSUM tile.
  P.T            one ACT exp op per (window, pair, k-chunk); fp16 out.
  O[q, d|sum]    TRANSPOSED P.V: stationary = P.T chunk [128k,128q],
                 moving = [V_h | ones] [128k, 65] -> accumulates
                 [128q, 65] in sub-bank PSUM slices (2x fewer PE rows
                 than the O.T orientation).  Lagged one (window,pair)
                 behind the exp stream.
  normalize      DVE reciprocal of col 64 + per-partition scalar mult.
  O.T            crossbar DMA transpose [128q,128hd] -> [128hd,128q].
  out[m, :]      stationary oT chunk [128 hd, 128 m] x moving Wo.T
                 [128 hd, 512 e], 2-chunk contraction, DVE copy + SWDGE
                 store.

dtypes: all matmul operands fp16 (1 cyc/row on PE); PSUM accumulation fp32;
softmax exp on fp32 scores.
"""

import numpy as np
from contextlib import ExitStack

import concourse.bass as bass
import concourse.mybir as mybir
import concourse.tile as tile
from concourse.tile import ScopedClock
from concourse.bass_utils import run_bass_kernel_spmd

# ---------------------------------------------------------------------------
# Workarounds for the walrus build on this stack, which rejects more than ONE
# semaphore wait per instruction ("Too many sync wait commands").
# ---------------------------------------------------------------------------
_orig_commit_instruction = tile.TileContext._commit_instruction


def _commit_instruction(self, inst, lazy_reg_writes=True):
    si = getattr(inst, "sync_info", None)
    if si is not None and si.on_wait and len(si.on_wait) > 1:
        waits = list(si.on_wait)
        for w in waits[:-1]:
            nop = mybir.InstNoOp(
                name=self.nc.get_next_instruction_name(),
                ins=[], outs=[], engine=inst.engine,
            )
            nop.bass_nofuse = True
            nop.sync_info = mybir.SyncInfo(on_wait=[w], on_update=[])
            _orig_commit_instruction(self, nop, lazy_reg_writes=False)
        inst.sync_info = mybir.SyncInfo(
            on_wait=[waits[-1]], on_update=list(si.on_update or [])
        )
    return _orig_commit_instruction(self, inst, lazy_reg_writes)


def _drain_and_barrier(self, tick_clock, wait_clock):
    nc = self.nc
    drain_inst = nc.sync.drain()
    wait_clock.add_sem_waits(
        drain_inst.ins, ScopedClock({None: tick_clock.global_clock})
    )
    si = drain_inst.ins.sync_info
    waits = list(si.on_wait) if si and si.on_wait else []
    if len(waits) > 1:
        drain_inst.ins.sync_info = mybir.SyncInfo(
            on_wait=waits[:1], on_update=list(si.on_update or [])
        )
        for w in waits[1:]:
            extra = nc.sync.drain()
            esi = extra.ins.sync_info
            extra.ins.sync_info = mybir.SyncInfo(
                on_wait=[w],
                on_update=list(esi.on_update or []) if esi else [],
            )
    nc.all_engine_barrier()
    assert self.sems is not None
    popped = nc._tile_sem_poison_stack.pop()
    assert popped is self._sem_poison
    nc.clear_and_free_semaphores(list(self.sems.allocated().values()))
    nc.all_engine_barrier()


def _apply_tilefix():
    tile.TileContext._commit_instruction = _commit_instruction
    tile.TileContext._drain_and_barrier = _drain_and_barrier


_apply_tilefix()

# ---------------------------------------------------------------------------
# Problem constants (hardcoded)
# ---------------------------------------------------------------------------
B, S, E, H = 2, 2048, 1024, 16
HC, D = 4, 64              # heads per core, head dim
NCORES = 8
NE = E // 128              # 8  e-chunks
NW = S // 512              # 4  q/k windows
NK = S // 128              # 16 k-chunks
NM = S // 128              # 16 m-chunks

F32 = mybir.dt.float32
FP16 = mybir.dt.float16


def build(ptbufs=18):
    nc = bass.Bass()
    xqT = nc.dram_tensor("xqT", [E, S], FP16, kind="ExternalInput")
    xkT = nc.dram_tensor("xkT", [E, S], FP16, kind="ExternalInput")
    xvT = nc.dram_tensor("xvT", [E, S], FP16, kind="ExternalInput")
    wqT = nc.dram_tensor("wqT", [E, 256], FP16, kind="ExternalInput")
    wkT = nc.dram_tensor("wkT", [E, 256], FP16, kind="ExternalInput")
    wvT = nc.dram_tensor("wvT", [E, 256], FP16, kind="ExternalInput")
    woT = nc.dram_tensor("woT", [256, E], FP16, kind="ExternalInput")
    out = nc.dram_tensor("out", [S, E], F32, kind="ExternalOutput")

    with tile.TileContext(nc) as tc, ExitStack() as ctx:
        consts = ctx.enter_context(tc.tile_pool(name="consts", bufs=1))
        wpool = ctx.enter_context(tc.tile_pool(name="w", bufs=1))
        actpool = ctx.enter_context(tc.tile_pool(name="acts", bufs=1))
        xkpool = ctx.enter_context(tc.tile_pool(name="xk", bufs=3))
        xqpool = ctx.enter_context(tc.tile_pool(name="xq", bufs=2))
        xvpool = ctx.enter_context(tc.tile_pool(name="xv", bufs=3))
        ptpool = ctx.enter_context(tc.tile_pool(name="pT", bufs=ptbufs))
        onpool = ctx.enter_context(tc.tile_pool(name="on", bufs=4))
        rpool = ctx.enter_context(tc.tile_pool(name="recip", bufs=8))
        opool = ctx.enter_context(tc.tile_pool(name="outstage", bufs=2))
        psS = ctx.enter_context(tc.tile_pool(name="psS", bufs=2, space="PSUM"))
        psOV = ctx.enter_context(tc.tile_pool(name="psOV", bufs=1, space="PSUM"))
        psX = ctx.enter_context(tc.tile_pool(name="psX", bufs=2, space="PSUM"))

        # preload the exp table before the hot loop
        dummy = consts.tile([1, 8], F32)
        nc.vector.memset(dummy[:], 0.0)
        nc.scalar.activation(dummy[:], dummy[:], mybir.ActivationFunctionType.Exp)

        wq_sb = wpool.tile([128, NE, 256], FP16)
        wk_sb = wpool.tile([128, NE, 256], FP16)
        wv_sb = wpool.tile([128, NE, 256], FP16)
        wo_sb = wpool.tile([128, 2, E], FP16)

        qT_sb = actpool.tile([128, 2, S], FP16)        # [(2 heads x d), pair, s]
        kT_sb = actpool.tile([128, 2, S], FP16)
        v_sb = actpool.tile([128, NK, HC, 65], FP16)   # [s%128, k, h, V_h|ones]
        oT_sb = actpool.tile([128, 2, S], FP16, name="oT")  # [(h2 d), pair, s]

        nc.vector.memset(v_sb[:, :, :, 64:65], 1.0)

        # ---- DMA emission order on the SP queue (arrival order == need) ---
        def colblock(x, j):
            return x[:, j * 512:(j + 1) * 512].rearrange(
                "(ec p) s -> p ec s", p=128)

        nc.sync.dma_start(wq_sb[:], wqT.rearrange("(ec p) n -> p ec n", p=128))
        xq_blks = {}
        xk_blks = {}
        xv_blks = {}

        def load_xq(j):
            t = xqpool.tile([128, NE, 512], FP16, tag="xqb", name=f"xq{j}")
            nc.sync.dma_start(t[:], colblock(xqT, j))
            xq_blks[j] = t

        def load_xk(j):
            t = xkpool.tile([128, NE, 512], FP16, tag="xkb", name=f"xk{j}")
            nc.sync.dma_start(t[:], colblock(xkT, j))
            xk_blks[j] = t

        def load_xv(j):
            t = xvpool.tile([128, NE, 512], FP16, tag="xvb", name=f"xv{j}")
            nc.sync.dma_start(t[:], colblock(xvT, j))
            xv_blks[j] = t

        load_xq(0)
        nc.sync.dma_start(wk_sb[:], wkT.rearrange("(ec p) n -> p ec n", p=128))
        for j in range(NW):
            load_xk(j)
        nc.sync.dma_start(wv_sb[:], wvT.rearrange("(ec p) n -> p ec n", p=128))
        load_xv(0)
        load_xv(1)
        load_xq(1)
        load_xv(2)
        load_xv(3)
        load_xq(2)
        load_xq(3)
        nc.sync.dma_start(wo_sb[:], woT.rearrange("(j p) e -> p j e", p=128))

        # ---- building blocks -------------------------------------------
        def wproj(w_sb, blk, dst, win):
            """project one 512-col window of x into dst[:, :, win] (2 psX)."""
            ws = slice(win * 512, (win + 1) * 512)
            for nch in range(2):
                ps = psX.tile([128, 512], F32, tag="px", name=f"pj{win}_{nch}")
                for e in range(NE):
                    nc.tensor.matmul(
                        ps[:],
                        w_sb[:, e, nch * 128:(nch + 1) * 128],
                        blk[:, e, :],
                        start=(e == 0), stop=(e == NE - 1))
                nc.vector.tensor_copy(dst[:, nch, ws], ps[:])

        def vproj(m):
            blk = xv_blks[m // 4]
            ps = psX.tile([128, 512], F32, tag="px", name=f"vp{m}")
            for e in range(NE):
                nc.tensor.matmul(
                    ps[:, 0:256],
                    blk[:, e, (m % 4) * 128:(m % 4 + 1) * 128],
                    wv_sb[:, e, :],
                    start=(e == 0), stop=(e == NE - 1))
            nc.vector.tensor_copy(
                v_sb[:, m, :, 0:64],
                ps[:, 0:256].rearrange("p (h c) -> p h c", h=HC))

        def ov_group(ovts, pair, pT, kc):
            """8 transposed-PV matmuls for one k-chunk; sub-bank psum accum."""
            for t_i, (ovt, qlocs) in enumerate(ovts):
                for si, (h2, ql) in enumerate(
                        [(h, q) for h in range(2) for q in qlocs]):
                    nc.tensor.matmul(
                        ovt[:, ql % 2, h2, :],
                        pT[:, h2 * 512 + ql * 128: h2 * 512 + (ql + 1) * 128],
                        v_sb[:, kc, 2 * pair + h2, :],
                        start=(kc == 0 and si == 0),
                        stop=(kc == NK - 1 and si == 3),
                        skip_group_check=True)

        def finalize(ovts, w, pair):
            """normalize + crossbar-transpose one (window, pair)."""
            for ovt, qlocs in ovts:
                for ql in qlocs:
                    o_n = onpool.tile([128, 128], FP16, tag="on")
                    for h2 in range(2):
                        rt = rpool.tile([128, 1], F32, tag="rt")
                        nc.vector.reciprocal(rt[:], ovt[:, ql % 2, h2, 64:65])
                        nc.vector.tensor_scalar_mul(
                            o_n[:, h2 * 64:(h2 + 1) * 64],
                            ovt[:, ql % 2, h2, 0:64],
                            rt[:])
                    qs = slice(w * 512 + ql * 128, w * 512 + (ql + 1) * 128)
                    nc.sync.dma_start_transpose(oT_sb[:, pair, qs], o_n[:])

        def outproj_half(m, j, stage):
            ps = psX.tile([128, 512], F32, tag="px", name=f"op{m}_{j}")
            for jp in range(2):
                nc.tensor.matmul(
                    ps[:],
                    oT_sb[:, jp, m * 128:(m + 1) * 128],
                    wo_sb[:, jp, j * 512:(j + 1) * 512],
                    start=(jp == 0), stop=(jp == 1))
            nc.vector.tensor_copy(stage[:, j * 512:(j + 1) * 512], ps[:])
            if j == 1:
                nc.gpsimd.dma_start(out[m * 128:(m + 1) * 128, :], stage[:])

        # ---- steady state ----------------------------------------------
        wproj(wq_sb, xq_blks[0], qT_sb, 0)
        wproj(wk_sb, xk_blks[0], kT_sb, 0)

        pts = {}
        prev_ovts = None
        stages = {}
        for p in range(8):
            w, pair = divmod(p, 2)
            qs = slice(w * 512, (w + 1) * 512)
            if p > 0:
                ovA = psOV.tile([128, 2, 2, 65], F32, tag="ovA")
                ovB = psOV.tile([128, 2, 2, 65], F32, tag="ovB")
                ovts = ((ovA, (0, 1)), (ovB, (2, 3)))
            # outproj window scheduled in this p (lag after transposes)
            opw = {3: 0, 5: 1, 7: 2}.get(p)

            for kc in range(NK):
                ks = slice(kc * 128, (kc + 1) * 128)
                ps_s = psS.tile([128, 1024], F32)
                nc.tensor.matmul(ps_s[:, 0:512],
                                 kT_sb[0:64, pair, ks],
                                 qT_sb[0:64, pair, qs],
                                 start=True, stop=True)
                nc.tensor.matmul(ps_s[:, 512:1024],
                                 kT_sb[64:128, pair, ks],
                                 qT_sb[64:128, pair, qs],
                                 start=True, stop=True)
                pT = ptpool.tile([128, 1024], FP16, tag="pT")
                nc.scalar.activation(pT[:], ps_s[:],
                                     mybir.ActivationFunctionType.Exp,
                                     scale=0.125)
                pts[(p, kc)] = pT

                # fillers behind this slot's scores
                if p == 0:
                    if kc in (1, 5, 9):
                        win = kc // 4 + 1
                        wproj(wk_sb, xk_blks[win], kT_sb, win)
                    if kc >= 8:
                        vproj(kc - 8)
                elif p == 1:
                    if kc < 8:
                        vproj(kc + 8)
                    if kc == 1:
                        wproj(wq_sb, xq_blks[1], qT_sb, 1)
                elif p == 3 and kc == 0:
                    wproj(wq_sb, xq_blks[2], qT_sb, 2)
                elif p == 5 and kc == 0:
                    wproj(wq_sb, xq_blks[3], qT_sb, 3)
                if opw is not None and kc % 2 == 1:
                    u = kc // 2          # 0..7 -> (m, j)
                    m = opw * 4 + u // 2
                    if u % 2 == 0:
                        stages[m] = opool.tile([128, E], F32, tag="ost", name=f"st{m}")
                    outproj_half(m, u % 2, stages[m])

                if p > 0:
                    pw, ppair = divmod(p - 1, 2)
                    ov_group(ovts, ppair, pts.pop((p - 1, kc)), kc)

            if p > 0:
                pw, ppair = divmod(p - 1, 2)
                finalize(ovts, pw, ppair)
                prev_ovts = ovts

        # ---- tail: OV + finalize for p=7, then outproj window 3 ---------
        ovA = psOV.tile([128, 2, 2, 65], F32, tag="ovA")
        ovB = psOV.tile([128, 2, 2, 65], F32, tag="ovB")
        ovts = ((ovA, (0, 1)), (ovB, (2, 3)))
        for kc in range(NK):
            ov_group(ovts, 1, pts.pop((7, kc)), kc)
        finalize(ovts, 3, 1)
        for m in range(12, 16):
            stage = opool.tile([128, E], F32, tag="ost", name=f"st{m}")
            outproj_half(m, 0, stage)
            outproj_half(m, 1, stage)

    return nc


_NC_CACHE = {}


def _get_nc():
    if "nc" not in _NC_CACHE:
        _NC_CACHE["nc"] = build()
    return _NC_CACHE["nc"]


def _shard_inputs(query, key, value, Wq, Wk, Wv, Wo):
    """Host-side sharding + layout prep: core c = (batch c//4, head-group c%4)."""
    f16 = np.float16
    xT = []
    for b in range(B):
        xT.append((
            np.ascontiguousarray(query[b].T).astype(f16),
            np.ascontiguousarray(key[b].T).astype(f16),
            np.ascontiguousarray(value[b].T).astype(f16),
        ))
    wT = []
    for g in range(4):
        gc = slice(g * 256, (g + 1) * 256)
        wT.append((
            np.ascontiguousarray(Wq[gc].T).astype(f16),
            np.ascontiguousarray(Wk[gc].T).astype(f16),
            np.ascontiguousarray(Wv[gc].T).astype(f16),
            np.ascontiguousarray(Wo[:, gc].T).astype(f16),
        ))
    in_maps = []
    for c in range(NCORES):
        b, g = c // 4, c % 4
        qT, kT, vT = xT[b]
        wq, wk, wv, wo = wT[g]
        in_maps.append({
            "xqT": qT, "xkT": kT, "xvT": vT,
            "wqT": wq, "wkT": wk, "wvT": wv, "woT": wo,
        })
    return in_maps


def kernel(query, key, value, Wq, Wk, Wv, Wo):
    query = np.asarray(query, dtype=np.float32)
    key = np.asarray(key, dtype=np.float32)
    value = np.asarray(value, dtype=np.float32)
    Wq = np.asarray(Wq, dtype=np.float32)
    Wk = np.asarray(Wk, dtype=np.float32)
    Wv = np.asarray(Wv, dtype=np.float32)
    Wo = np.asarray(Wo, dtype=np.float32)

    nc = _get_nc()
    in_maps = _shard_inputs(query, key, value, Wq, Wk, Wv, Wo)
    res = run_bass_kernel_spmd(nc, in_maps, core_ids=list(range(NCORES)))

    out = np.zeros((B, S, E), dtype=np.float32)
    for c in range(NCORES):
        out[c // 4] += res.results[c]["out"]
    return out


# revision 7
# speedup vs baseline: 1.3719x; 1.0100x over previous
"""Trainium2 Bass kernel for nn_MultiHeadAttention (B=2, S=2048, E=1024, H=16).

Sharding: 8 NeuronCores = data-parallel over the 2 batches x tensor-parallel
over the 16 heads in 4 groups of 4 heads (Wq/Wk/Wv split column-wise, Wo
row-wise).  Each core computes a full-[S, E] partial of its batch's output;
the host sums the 4 head-group partials per batch.

Per-core pipeline (ACT exp stream is the pace-setter at ~133us):
  Q.T/K.T[n, s]  per 512-col window: psX-accumulated e-outer matmuls
                 chasing column-block DMAs (first exp at ~11us).
  S.T_h[k, q]    [64,128]x[64,512] per (window, head-pair, k-chunk),
                 head pair packed in one [128,1024] PSUM tile.
  P.T            one ACT exp op per (window, pair, k-chunk); fp16 out.
  O[q, d|sum]    TRANSPOSED P.V: stationary = P.T chunk [128k,128q],
                 moving = [V_h | ones] [128k, 65] -> accumulates
                 [128q, 65] in sub-bank PSUM slices (2x fewer PE rows
                 than the O.T orientation).  Lagged one (window,pair)
                 behind the exp stream.
  normalize      DVE reciprocal of col 64 + per-partition scalar mult.
  O.T            crossbar DMA transpose [128q,128hd] -> [128hd,128q].
  out[m, :]      stationary oT chunk [128 hd, 128 m] x moving Wo.T
                 [128 hd, 512 e], 2-chunk contraction, DVE copy + SWDGE
                 store.

dtypes: all matmul operands fp16 (1 cyc/row on PE); PSUM accumulation fp32;
softmax exp on fp32 scores.
"""

import numpy as np
from contextlib import ExitStack

import concourse.bass as bass
import concourse.mybir as mybir
import concourse.tile as tile
from concourse.tile import ScopedClock
from concourse.bass_utils import run_bass_kernel_spmd

# ---------------------------------------------------------------------------
# Workarounds for the walrus build on this stack, which rejects more than ONE
# semaphore wait per instruction ("Too many sync wait commands").
# ---------------------------------------------------------------------------
_orig_commit_instruction = tile.TileContext._commit_instruction


def _commit_instruction(self, inst, lazy_reg_writes=True):
    si = getattr(inst, "sync_info", None)
    if si is not None and si.on_wait and len(si.on_wait) > 1:
        waits = list(si.on_wait)
        for w in waits[:-1]:
            nop = mybir.InstNoOp(
                name=self.nc.get_next_instruction_name(),
                ins=[], outs=[], engine=inst.engine,
            )
            nop.bass_nofuse = True
            nop.sync_info = mybir.SyncInfo(on_wait=[w], on_update=[])
            _orig_commit_instruction(self, nop, lazy_reg_writes=False)
        inst.sync_info = mybir.SyncInfo(
            on_wait=[waits[-1]], on_update=list(si.on_update or [])
        )
    return _orig_commit_instruction(self, inst, lazy_reg_writes)


def _drain_and_barrier(self, tick_clock, wait_clock):
    nc = self.nc
    drain_inst = nc.sync.drain()
    wait_clock.add_sem_waits(
        drain_inst.ins, ScopedClock({None: tick_clock.global_clock})
    )
    si = drain_inst.ins.sync_info
    waits = list(si.on_wait) if si and si.on_wait else []
    if len(waits) > 1:
        drain_inst.ins.sync_info = mybir.SyncInfo(
            on_wait=waits[:1], on_update=list(si.on_update or [])
        )
        for w in waits[1:]:
            extra = nc.sync.drain()
            esi = extra.ins.sync_info
            extra.ins.sync_info = mybir.SyncInfo(
                on_wait=[w],
                on_update=list(esi.on_update or []) if esi else [],
            )
    nc.all_engine_barrier()
    assert self.sems is not None
    popped = nc._tile_sem_poison_stack.pop()
    assert popped is self._sem_poison
    nc.clear_and_free_semaphores(list(self.sems.allocated().values()))
    nc.all_engine_barrier()


def _apply_tilefix():
    tile.TileContext._commit_instruction = _commit_instruction
    tile.TileContext._drain_and_barrier = _drain_and_barrier


_apply_tilefix()

# ---------------------------------------------------------------------------
# Problem constants (hardcoded)
# ---------------------------------------------------------------------------
B, S, E, H = 2, 2048, 1024, 16
HC, D = 4, 64              # heads per core, head dim
NCORES = 8
NE = E // 128              # 8  e-chunks
NW = S // 512              # 4  q/k windows
NK = S // 128              # 16 k-chunks
NM = S // 128              # 16 m-chunks

F32 = mybir.dt.float32
FP16 = mybir.dt.float16


def build(ptbufs=18):
    nc = bass.Bass()
    xqT = nc.dram_tensor("xqT", [E, S], FP16, kind="ExternalInput")
    xkT = nc.dram_tensor("xkT", [E, S], FP16, kind="ExternalInput")
    xvT = nc.dram_tensor("xvT", [E, S], FP16, kind="ExternalInput")
    wqT = nc.dram_tensor("wqT", [E, 256], FP16, kind="ExternalInput")
    wkT = nc.dram_tensor("wkT", [E, 256], FP16, kind="ExternalInput")
    wvT = nc.dram_tensor("wvT", [E, 256], FP16, kind="ExternalInput")
    woT = nc.dram_tensor("woT", [256, E], FP16, kind="ExternalInput")
    out = nc.dram_tensor("out", [S, E], F32, kind="ExternalOutput")

    with tile.TileContext(nc) as tc, ExitStack() as ctx:
        consts = ctx.enter_context(tc.tile_pool(name="consts", bufs=1))
        wpool = ctx.enter_context(tc.tile_pool(name="w", bufs=1))
        actpool = ctx.enter_context(tc.tile_pool(name="acts", bufs=1))
        xkpool = ctx.enter_context(tc.tile_pool(name="xk", bufs=3))
        xqpool = ctx.enter_context(tc.tile_pool(name="xq", bufs=3))
        xvpool = ctx.enter_context(tc.tile_pool(name="xv", bufs=3))
        ptpool = ctx.enter_context(tc.tile_pool(name="pT", bufs=ptbufs))
        onpool = ctx.enter_context(tc.tile_pool(name="on", bufs=4))
        rpool = ctx.enter_context(tc.tile_pool(name="recip", bufs=8))
        opool = ctx.enter_context(tc.tile_pool(name="outstage", bufs=2))
        psS = ctx.enter_context(tc.tile_pool(name="psS", bufs=2, space="PSUM"))
        psOV = ctx.enter_context(tc.tile_pool(name="psOV", bufs=1, space="PSUM"))
        psX = ctx.enter_context(tc.tile_pool(name="psX", bufs=2, space="PSUM"))

        # preload the exp table before the hot loop
        dummy = consts.tile([1, 8], F32)
        nc.vector.memset(dummy[:], 0.0)
        nc.scalar.activation(dummy[:], dummy[:], mybir.ActivationFunctionType.Exp)

        wq_sb = wpool.tile([128, NE, 256], FP16)
        wk_sb = wpool.tile([128, NE, 256], FP16)
        wv_sb = wpool.tile([128, NE, 256], FP16)
        wo_sb = wpool.tile([128, 2, E], FP16)

        qT_sb = actpool.tile([128, 2, S], FP16)        # [(2 heads x d), pair, s]
        kT_sb = actpool.tile([128, 2, S], FP16)
        v_sb = actpool.tile([128, NK, HC, 65], FP16)   # [s%128, k, h, V_h|ones]
        oT_sb = actpool.tile([128, 2, S], FP16, name="oT")  # [(h2 d), pair, s]

        nc.vector.memset(v_sb[:, :, :, 64:65], 1.0)

        # ---- DMA emission order on the SP queue (arrival order == need) ---
        def colblock(x, j):
            return x[:, j * 512:(j + 1) * 512].rearrange(
                "(ec p) s -> p ec s", p=128)

        def halfblock(x, j, h):
            return x[h * 512:(h + 1) * 512,
                     j * 512:(j + 1) * 512].rearrange(
                "(ec p) s -> p ec s", p=128)

        xq_blks = {}
        xk_blks = {}
        xv_blks = {}

        def load_x(pool, src, blks, j, tag, split=False):
            t = pool.tile([128, NE, 512], FP16, tag=tag, name=f"{tag}{j}")
            if split:
                # two half-e DMAs so the projection can chase the first half
                nc.sync.dma_start(t[:, 0:4, :], halfblock(src, j, 0))
                nc.sync.dma_start(t[:, 4:8, :], halfblock(src, j, 1))
            else:
                nc.sync.dma_start(t[:], colblock(src, j))
            blks[j] = t

        # arrival order == need order (single serialized DMA device)
        nc.sync.dma_start(wk_sb[:], wkT.rearrange("(ec p) n -> p ec n", p=128))
        load_x(xkpool, xkT, xk_blks, 0, "xkb", split=True)
        nc.sync.dma_start(wq_sb[:], wqT.rearrange("(ec p) n -> p ec n", p=128))
        load_x(xqpool, xqT, xq_blks, 0, "xqb", split=True)
        for j in range(1, NW):
            load_x(xkpool, xkT, xk_blks, j, "xkb")
        nc.sync.dma_start(wv_sb[:], wvT.rearrange("(ec p) n -> p ec n", p=128))
        for j in range(NW):
            load_x(xvpool, xvT, xv_blks, j, "xvb")
        for j in range(1, NW):
            load_x(xqpool, xqT, xq_blks, j, "xqb")
        nc.sync.dma_start(wo_sb[:], woT.rearrange("(j p) e -> p j e", p=128))

        # ---- building blocks -------------------------------------------
        def wproj(w_sb, blk, dst, win):
            """project one 512-col window of x into dst[:, :, win] (2 psX)."""
            ws = slice(win * 512, (win + 1) * 512)
            for nch in range(2):
                ps = psX.tile([128, 512], F32, tag="px", name=f"pj{win}_{nch}")
                for e in range(NE):
                    nc.tensor.matmul(
                        ps[:],
                        w_sb[:, e, nch * 128:(nch + 1) * 128],
                        blk[:, e, :],
                        start=(e == 0), stop=(e == NE - 1))
                nc.vector.tensor_copy(dst[:, nch, ws], ps[:])

        def vproj(m):
            blk = xv_blks[m // 4]
            ps = psX.tile([128, 512], F32, tag="px", name=f"vp{m}")
            for e in range(NE):
                nc.tensor.matmul(
                    ps[:, 0:256],
                    blk[:, e, (m % 4) * 128:(m % 4 + 1) * 128],
                    wv_sb[:, e, :],
                    start=(e == 0), stop=(e == NE - 1))
            nc.vector.tensor_copy(
                v_sb[:, m, :, 0:64],
                ps[:, 0:256].rearrange("p (h c) -> p h c", h=HC))

        def ov_group(ovts, pair, pT, kc):
            """8 transposed-PV matmuls for one k-chunk; sub-bank psum accum."""
            for t_i, (ovt, qlocs) in enumerate(ovts):
                for si, (h2, ql) in enumerate(
                        [(h, q) for h in range(2) for q in qlocs]):
                    nc.tensor.matmul(
                        ovt[:, ql % 2, h2, :],
                        pT[:, h2 * 512 + ql * 128: h2 * 512 + (ql + 1) * 128],
                        v_sb[:, kc, 2 * pair + h2, :],
                        start=(kc == 0 and si == 0),
                        stop=(kc == NK - 1 and si == 3),
                        skip_group_check=True)

        def finalize(ovts, w, pair):
            """normalize + crossbar-transpose one (window, pair)."""
            for ovt, qlocs in ovts:
                for ql in qlocs:
                    o_n = onpool.tile([128, 128], FP16, tag="on")
                    for h2 in range(2):
                        rt = rpool.tile([128, 1], F32, tag="rt")
                        nc.vector.reciprocal(rt[:], ovt[:, ql % 2, h2, 64:65])
                        nc.vector.tensor_scalar_mul(
                            o_n[:, h2 * 64:(h2 + 1) * 64],
                            ovt[:, ql % 2, h2, 0:64],
                            rt[:])
                    qs = slice(w * 512 + ql * 128, w * 512 + (ql + 1) * 128)
                    nc.sync.dma_start_transpose(oT_sb[:, pair, qs], o_n[:])

        def outproj_half(m, j, stage):
            ps = psX.tile([128, 512], F32, tag="px", name=f"op{m}_{j}")
            for jp in range(2):
                nc.tensor.matmul(
                    ps[:],
                    oT_sb[:, jp, m * 128:(m + 1) * 128],
                    wo_sb[:, jp, j * 512:(j + 1) * 512],
                    start=(jp == 0), stop=(jp == 1))
            nc.vector.tensor_copy(stage[:, j * 512:(j + 1) * 512], ps[:])
            if j == 1:
                nc.gpsimd.dma_start(out[m * 128:(m + 1) * 128, :], stage[:])

        # ---- global-slot schedule --------------------------------------
        # slot g = p*16 + kc carries scores(p,kc)+exp; OV work is lagged
        # OVLAG slots behind the exp stream (rolling across p boundaries).
        OVLAG = 10
        from collections import defaultdict
        extras_pre = defaultdict(list)    # g -> thunks (feeders: proj work)
        extras_post = defaultdict(list)   # g -> thunks (drains: outproj)

        def sched_wproj(g, w_sb, blks, dst, win, nch=None):
            for n in ((0, 1) if nch is None else (nch,)):
                extras_pre[g].append(
                    lambda n=n, win=win: wproj1(w_sb, blks[win], dst, win, n))

        def wproj1(w_sb, blk, dst, win, nch):
            ws = slice(win * 512, (win + 1) * 512)
            ps = psX.tile([128, 512], F32, tag="px", name=f"pj{win}_{nch}")
            for e in range(NE):
                nc.tensor.matmul(
                    ps[:],
                    w_sb[:, e, nch * 128:(nch + 1) * 128],
                    blk[:, e, :],
                    start=(e == 0), stop=(e == NE - 1))
            nc.vector.tensor_copy(dst[:, nch, ws], ps[:])

        # K windows 1-3 early in p0 (chasing the xk block DMAs)
        sched_wproj(0, wk_sb, xk_blks, kT_sb, 1)
        sched_wproj(4, wk_sb, xk_blks, kT_sb, 2)
        sched_wproj(8, wk_sb, xk_blks, kT_sb, 3)
        # V tiles 2 slots ahead of their OV consumer
        for m in range(NM):
            extras_pre[m + 8].append(lambda m=m: vproj(m))
        # remaining Q windows, off the critical path
        sched_wproj(24, wq_sb, xq_blks, qT_sb, 1, nch=0)
        sched_wproj(26, wq_sb, xq_blks, qT_sb, 1, nch=1)
        sched_wproj(56, wq_sb, xq_blks, qT_sb, 2)
        sched_wproj(88, wq_sb, xq_blks, qT_sb, 3)

        # outproj window w' after both its finalizes: units u0..3 at
        # p(2w'+2) slots 12..15, u4..7 at p(2w'+3) slots 0,2,4,6
        stages = {}

        def outproj_unit(opw, u):
            m = opw * 4 + u // 2
            if u % 2 == 0:
                stages[m] = opool.tile([128, E], F32, tag="ost", name=f"st{m}")
            outproj_half(m, u % 2, stages[m])

        for opw in range(3):
            for u in range(4):
                extras_post[(2 * opw + 2) * 16 + 12 + u].append(
                    lambda opw=opw, u=u: outproj_unit(opw, u))
            for u in range(4, 8):
                extras_post[(2 * opw + 3) * 16 + 2 * (u - 4)].append(
                    lambda opw=opw, u=u: outproj_unit(opw, u))

        # OV emission: n-th group (n = pp*16+pkc) at slot n+OVLAG, except
        # pp=7 doubled up at slots 122..127 to shrink the tail.
        ovmap = defaultdict(list)
        for n in range(0, 112):
            ovmap[n + OVLAG].append(n)
        for i, g in enumerate(range(122, 128)):
            ovmap[g] += [112 + 2 * i, 113 + 2 * i]
        # finalize(pp) goes right before ov(pp+1, 0) is first emitted
        finmap = {pp * 16 + 16 + OVLAG: pp for pp in range(6)}
        finmap[122] = 6

        # ---- warmup: anchor the PE p-state ramp while DMAs stream ------
        wu_a = consts.tile([128, 128], FP16)
        wu_b = consts.tile([128, 512], FP16)
        nc.vector.memset(wu_a[:], 0.0)
        nc.vector.memset(wu_b[:], 0.0)
        for i in range(8):
            ps = psX.tile([128, 512], F32, tag="px", name=f"wu{i}")
            nc.tensor.matmul(ps[:], wu_a[:], wu_b[:], start=True, stop=True)

        wproj1(wk_sb, xk_blks[0], kT_sb, 0, 0)
        wproj1(wk_sb, xk_blks[0], kT_sb, 0, 1)
        wproj1(wq_sb, xq_blks[0], qT_sb, 0, 0)
        wproj1(wq_sb, xq_blks[0], qT_sb, 0, 1)

        pts = {}
        ovts = None
        for g in range(128):
            p, kc = divmod(g, 16)
            w, pair = divmod(p, 2)
            qs = slice(w * 512, (w + 1) * 512)
            ks = slice(kc * 128, (kc + 1) * 128)

            ps_s = psS.tile([128, 1024], F32)
            nc.tensor.matmul(ps_s[:, 0:512],
                             kT_sb[0:64, pair, ks],
                             qT_sb[0:64, pair, qs],
                             start=True, stop=True)
            nc.tensor.matmul(ps_s[:, 512:1024],
                             kT_sb[64:128, pair, ks],
                             qT_sb[64:128, pair, qs],
                             start=True, stop=True)
            pT = ptpool.tile([128, 1024], FP16, tag="pT")
            nc.scalar.activation(pT[:], ps_s[:],
                                 mybir.ActivationFunctionType.Exp,
                                 scale=0.125)
            pts[g] = pT

            for thunk in extras_pre.get(g, ()):
                thunk()
            if g in finmap:
                pp = finmap[g]
                finalize(ovts, pp // 2, pp % 2)
            for n in ovmap.get(g, ()):
                pp, pkc = divmod(n, 16)
                if pkc == 0:
                    ovA = psOV.tile([128, 2, 2, 65], F32, tag="ovA",
                                    name=f"ovA{pp}")
                    ovB = psOV.tile([128, 2, 2, 65], F32, tag="ovB",
                                    name=f"ovB{pp}")
                    ovts = ((ovA, (0, 1)), (ovB, (2, 3)))
                ov_group(ovts, pp % 2, pts.pop(n), pkc)
            for thunk in extras_post.get(g, ()):
                thunk()

        # ---- tail: OV(7, 12..15), finalize(7) + outproj(w3) pipelined ---
        for n in range(124, 128):
            ov_group(ovts, 1, pts.pop(n), n % 16)
        for ql in range(4):
            ovt = ovts[0] if ql < 2 else ovts[1]
            o_n = onpool.tile([128, 128], FP16, tag="on")
            for h2 in range(2):
                rt = rpool.tile([128, 1], F32, tag="rt")
                nc.vector.reciprocal(rt[:], ovt[0][:, ql % 2, h2, 64:65])
                nc.vector.tensor_scalar_mul(
                    o_n[:, h2 * 64:(h2 + 1) * 64],
                    ovt[0][:, ql % 2, h2, 0:64],
                    rt[:])
            qs = slice(3 * 512 + ql * 128, 3 * 512 + (ql + 1) * 128)
            nc.sync.dma_start_transpose(oT_sb[:, 1, qs], o_n[:])
            m = 12 + ql
            stage = opool.tile([128, E], F32, tag="ost", name=f"st{m}")
            outproj_half(m, 0, stage)
            outproj_half(m, 1, stage)

    return nc


_NC_CACHE = {}


def _get_nc():
    if "nc" not in _NC_CACHE:
        _NC_CACHE["nc"] = build()
    return _NC_CACHE["nc"]


def _shard_inputs(query, key, value, Wq, Wk, Wv, Wo):
    """Host-side sharding + layout prep: core c = (batch c//4, head-group c%4)."""
    f16 = np.float16
    xT = []
    for b in range(B):
        xT.append((
            np.ascontiguousarray(query[b].T).astype(f16),
            np.ascontiguousarray(key[b].T).astype(f16),
            np.ascontiguousarray(value[b].T).astype(f16),
        ))
    wT = []
    for g in range(4):
        gc = slice(g * 256, (g + 1) * 256)
        wT.append((
            np.ascontiguousarray(Wq[gc].T).astype(f16),
            np.ascontiguousarray(Wk[gc].T).astype(f16),
            np.ascontiguousarray(Wv[gc].T).astype(f16),
            np.ascontiguousarray(Wo[:, gc].T).astype(f16),
        ))
    in_maps = []
    for c in range(NCORES):
        b, g = c // 4, c % 4
        qT, kT, vT = xT[b]
        wq, wk, wv, wo = wT[g]
        in_maps.append({
            "xqT": qT, "xkT": kT, "xvT": vT,
            "wqT": wq, "wkT": wk, "wvT": wv, "woT": wo,
        })
    return in_maps


def kernel(query, key, value, Wq, Wk, Wv, Wo):
    query = np.asarray(query, dtype=np.float32)
    key = np.asarray(key, dtype=np.float32)
    value = np.asarray(value, dtype=np.float32)
    Wq = np.asarray(Wq, dtype=np.float32)
    Wk = np.asarray(Wk, dtype=np.float32)
    Wv = np.asarray(Wv, dtype=np.float32)
    Wo = np.asarray(Wo, dtype=np.float32)

    nc = _get_nc()
    in_maps = _shard_inputs(query, key, value, Wq, Wk, Wv, Wo)
    res = run_bass_kernel_spmd(nc, in_maps, core_ids=list(range(NCORES)))

    out = np.zeros((B, S, E), dtype=np.float32)
    for c in range(NCORES):
        out[c // 4] += res.results[c]["out"]
    return out
